# revision 5
# baseline (speedup 1.0000x reference)
"""Trainium2 Bass kernel: GQA multi-head attention (B=1, S=2048, D=2048,
16 query heads, 4 KV heads, causal) sharded over 8 NeuronCores.

Sharding: tensor-parallel over heads. Core c owns query heads {2c, 2c+1}
and KV head c//2. Each core computes its Q/K/V projections, causal
attention for its 2 heads, and a partial output projection through its
256 rows of Wo^T. The host sums the 8 partial [S, D] outputs and adds bo.

Layout notes (per core):
  - x is fed transposed (xT [D, S], bf16) so projections produce
    Q^T/K^T [dk, S] directly (lhsT = W^T chunk, rhs = xT chunk).
  - V is produced in natural [S, dk] layout (lhsT = xT chunk, rhs = Wv^T).
  - Attention runs in transposed layout: scores^T[k, q] = K^T_tile.T @ Q^T,
    P^T = exp(scale * scores^T) (no max subtraction; scores are O(+-8)
    for this problem's distribution), row sums via an all-ones matmul on
    the tensor engine (broadcast across partitions), normalization folded
    into the PSUM eviction of attnout^T.
  - Causal masking: fully-masked 512-wide key/query blocks are skipped,
    diagonal blocks get a narrowed free dim plus a 0/1 mask multiply.
"""

import sys

if "/opt/trn_rl_repo" not in sys.path:
    sys.path.insert(0, "/opt/trn_rl_repo")

from contextlib import ExitStack

import numpy as np
import ml_dtypes

D_MODEL = 2048
S = 2048
NUM_HEADS = 16
GROUP = 4
NUM_KV = NUM_HEADS // GROUP  # 4
DK = D_MODEL // NUM_HEADS  # 128
N_CORES = 8
HPC = NUM_HEADS // N_CORES  # 2 query heads per core
KV_DIM = DK * NUM_KV  # 512
SCALE = 1.0 / float(np.sqrt(DK))
BF16 = ml_dtypes.bfloat16

NJ = D_MODEL // 128  # 16 contraction chunks
NSC = S // 512  # 4 query chunks of 512
NST = S // 128  # 16 s-tiles / k-tiles

_CACHE: dict = {}


def _build_nc(n_iters: int = 1):
    import concourse.bass as bass
    from concourse import bacc, tile, mybir

    f32 = mybir.dt.float32
    bf16 = mybir.dt.bfloat16

    nc = bacc.Bacc("TRN2", target_bir_lowering=False, debug=False,
                   num_devices=N_CORES)

    xT_d = nc.dram_tensor("xT", [D_MODEL, S], bf16, kind="ExternalInput")
    wqT_d = nc.dram_tensor("wqT", [D_MODEL, HPC * DK], bf16, kind="ExternalInput")
    wkT_d = nc.dram_tensor("wkT", [D_MODEL, DK], bf16, kind="ExternalInput")
    wvT_d = nc.dram_tensor("wvT", [D_MODEL, DK], bf16, kind="ExternalInput")
    woT_d = nc.dram_tensor("woT", [HPC * DK, D_MODEL], bf16, kind="ExternalInput")
    bq_d = nc.dram_tensor("bq", [HPC * DK, 1], f32, kind="ExternalInput")
    bk_d = nc.dram_tensor("bk", [DK, 1], f32, kind="ExternalInput")
    bvb_d = nc.dram_tensor("bvb", [128, DK], f32, kind="ExternalInput")
    masks_d = nc.dram_tensor("masks", [4, 128, 512], bf16, kind="ExternalInput")
    y_d = nc.dram_tensor("y", [S, D_MODEL], f32, kind="ExternalOutput")

    with tile.TileContext(nc) as tc, ExitStack() as ctx:
        const = ctx.enter_context(tc.tile_pool(name="const", bufs=1))
        big = ctx.enter_context(tc.tile_pool(name="big", bufs=1))
        pt_pool = ctx.enter_context(tc.tile_pool(name="pt", bufs=4))
        recip_pool = ctx.enter_context(tc.tile_pool(name="recip", bufs=2))
        yev_pool = ctx.enter_context(tc.tile_pool(name="yev", bufs=3))
        ps_qk = ctx.enter_context(
            tc.tile_pool(name="ps_qk", bufs=2, space=bass.MemorySpace.PSUM))
        ps_v = ctx.enter_context(
            tc.tile_pool(name="ps_v", bufs=1, space=bass.MemorySpace.PSUM))
        ps_sc = ctx.enter_context(
            tc.tile_pool(name="ps_sc", bufs=2, space=bass.MemorySpace.PSUM))
        ps_sum = ctx.enter_context(
            tc.tile_pool(name="ps_sum", bufs=1, space=bass.MemorySpace.PSUM))
        ps_av = ctx.enter_context(
            tc.tile_pool(name="ps_av", bufs=2, space=bass.MemorySpace.PSUM))

        if n_iters > 1:
            hint = (mybir.EngineType.PE, mybir.EngineType.Activation,
                    mybir.EngineType.DVE, mybir.EngineType.SP)
            ctx.enter_context(tc.For_i(0, n_iters, 1, hint_engines=hint))
        if True:
            # ---- constants / weights into SBUF ----
            wq_sb = const.tile([128, NJ, HPC * DK], bf16, tag="wq")
            wk_sb = const.tile([128, NJ, DK], bf16, tag="wk")
            wv_sb = const.tile([128, NJ, DK], bf16, tag="wv")
            wo_sb = const.tile([128, HPC, D_MODEL], bf16, tag="wo")
            masks_sb = const.tile([128, 4, 512], bf16, tag="masks")
            ones_sb = const.tile([128, 128], bf16, tag="ones")
            bq_sb = const.tile([128, HPC, 1], f32, tag="bq")
            bk_sb = const.tile([128, 1], f32, tag="bk")
            bvb_sb = const.tile([128, DK], f32, tag="bvb")

            wqT_r = wqT_d[:].rearrange("(j p) d -> p j d", p=128)
            wkT_r = wkT_d[:].rearrange("(j p) d -> p j d", p=128)
            wvT_r = wvT_d[:].rearrange("(j p) d -> p j d", p=128)
            for j in range(NJ):
                nc.sync.dma_start(out=wq_sb[:, j, :], in_=wqT_r[:, j, :])
                nc.sync.dma_start(out=wk_sb[:, j, :], in_=wkT_r[:, j, :])
                nc.sync.dma_start(out=wv_sb[:, j, :], in_=wvT_r[:, j, :])
            nc.sync.dma_start(
                out=wo_sb[:], in_=woT_d[:].rearrange("(h p) e -> p h e", p=128))
            nc.sync.dma_start(
                out=masks_sb[:], in_=masks_d[:].rearrange("r p q -> p r q"))
            nc.vector.memset(ones_sb[:], 1.0)
            nc.sync.dma_start(
                out=bq_sb[:], in_=bq_d[:].rearrange("(h p) o -> p h o", p=128))
            nc.sync.dma_start(out=bk_sb[:], in_=bk_d[:])
            nc.sync.dma_start(out=bvb_sb[:], in_=bvb_d[:])

            xT_sb = big.tile([128, NJ, S], bf16, tag="xT")
            for j in range(NJ):
                nc.sync.dma_start(
                    out=xT_sb[:, j, :], in_=xT_d[j * 128:(j + 1) * 128, :])

            qT_sb = big.tile([128, HPC, S], bf16, tag="qT")
            kT_sb = big.tile([128, S], bf16, tag="kT")
            v_sb = big.tile([128, NST, DK], bf16, tag="v")
            attnT_sb = big.tile([128, HPC, S], bf16, tag="attnT")

            # ---- phase 1: projections ----
            # Emit per s-chunk so attention on early chunks can start ASAP.
            for sc in range(NSC):
                s_lo, s_hi = sc * 512, (sc + 1) * 512
                for h in range(HPC):
                    qps = ps_qk.tile([128, 512], f32, tag="qk")
                    for j in range(NJ):
                        nc.tensor.matmul(
                            qps[:],
                            wq_sb[:, j, h * DK:(h + 1) * DK],
                            xT_sb[:, j, s_lo:s_hi],
                            start=(j == 0), stop=(j == NJ - 1))
                    nc.vector.tensor_scalar_add(
                        out=qT_sb[:, h, s_lo:s_hi], in0=qps[:],
                        scalar1=bq_sb[:, h, :])
                kps = ps_qk.tile([128, 512], f32, tag="qk")
                for j in range(NJ):
                    nc.tensor.matmul(
                        kps[:], wk_sb[:, j, :], xT_sb[:, j, s_lo:s_hi],
                        start=(j == 0), stop=(j == NJ - 1))
                nc.vector.tensor_scalar_add(
                    out=kT_sb[:, s_lo:s_hi], in0=kps[:], scalar1=bk_sb[:])
                for st in range(sc * 4, sc * 4 + 4):
                    vps = ps_v.tile([128, DK], f32, tag="v")
                    for j in range(NJ):
                        nc.tensor.matmul(
                            vps[:],
                            xT_sb[:, j, st * 128:(st + 1) * 128],
                            wv_sb[:, j, :],
                            start=(j == 0), stop=(j == NJ - 1))
                    nc.vector.tensor_add(
                        out=v_sb[:, st, :], in0=vps[:], in1=bvb_sb[:])

            # ---- phase 2+3: attention, then partial Wo per query chunk ----
            for qc in range(NSC):
                q_lo = qc * 512
                nkt = 4 * qc + 4  # k-tiles 0 .. 4qc+3 (rest fully masked)
                for h in range(HPC):
                    avps = ps_av.tile([128, 512], f32, tag="av")
                    sps = ps_sum.tile([128, 512], f32, tag="sum")
                    for kt in range(nkt):
                        r = kt - 4 * qc  # >=0 on diagonal blocks
                        off = 128 * r if r > 0 else 0
                        scps = ps_sc.tile([128, 512], f32, tag="sc")
                        nc.tensor.matmul(
                            scps[:, off:512],
                            kT_sb[:, kt * 128:(kt + 1) * 128],
                            qT_sb[:, h, q_lo + off:q_lo + 512],
                            start=True, stop=True)
                        pt = pt_pool.tile([128, 512], bf16, tag="pt")
                        nc.scalar.activation(
                            out=pt[:, off:512], in_=scps[:, off:512],
                            func=mybir.ActivationFunctionType.Exp,
                            scale=SCALE)
                        if r >= 0:
                            nc.vector.tensor_mul(
                                out=pt[:, off:512], in0=pt[:, off:512],
                                in1=masks_sb[:, r, off:512])
                        nc.tensor.matmul(
                            sps[:, off:512], ones_sb[:], pt[:, off:512],
                            start=(kt == 0), stop=(kt == nkt - 1),
                            skip_group_check=True)
                        nc.tensor.matmul(
                            avps[:, off:512], v_sb[:, kt, :], pt[:, off:512],
                            start=(kt == 0), stop=(kt == nkt - 1),
                            skip_group_check=True)
                    recip = recip_pool.tile([128, 512], f32, tag="recip")
                    nc.vector.reciprocal(out=recip[:], in_=sps[:])
                    nc.vector.tensor_mul(
                        out=attnT_sb[:, h, q_lo:q_lo + 512], in0=avps[:],
                        in1=recip[:])
                # partial output projection for this chunk's 4 s-tiles
                for st in range(qc * 4, qc * 4 + 4):
                    for ec in range(4):
                        yps = ps_qk.tile([128, 512], f32, tag="qk")
                        for h in range(HPC):
                            nc.tensor.matmul(
                                yps[:],
                                attnT_sb[:, h, st * 128:(st + 1) * 128],
                                wo_sb[:, h, ec * 512:(ec + 1) * 512],
                                start=(h == 0), stop=(h == HPC - 1))
                        ysb = yev_pool.tile([128, 512], f32, tag="yev")
                        if ec % 2 == 0:
                            nc.scalar.copy(out=ysb[:], in_=yps[:])
                        else:
                            nc.vector.tensor_copy(out=ysb[:], in_=yps[:])
                        nc.sync.dma_start(
                            out=y_d[st * 128:(st + 1) * 128,
                                    ec * 512:(ec + 1) * 512],
                            in_=ysb[:])

    nc.compile()
    return nc


def _get_nc(n_iters: int = 1):
    key = ("nc", n_iters)
    if key not in _CACHE:
        _CACHE[key] = _build_nc(n_iters)
    return _CACHE[key]


def _make_masks() -> np.ndarray:
    kk = np.arange(128)[:, None]
    qq = np.arange(512)[None, :]
    masks = np.zeros((4, 128, 512), dtype=np.float32)
    for r in range(4):
        masks[r] = (128 * r + kk <= qq).astype(np.float32)
    return masks.astype(BF16)


def _prep_in_maps(x, Wq, bq, Wk, bk, Wv, bv, Wo, bo):
    x = np.asarray(x, dtype=np.float32)
    xT = np.ascontiguousarray(x.reshape(S, D_MODEL).T).astype(BF16)
    masks = _make_masks()
    in_maps = []
    for c in range(N_CORES):
        kv = c // 2
        q_rows = slice(c * HPC * DK, (c + 1) * HPC * DK)
        kv_rows = slice(kv * DK, (kv + 1) * DK)
        in_maps.append({
            "xT": xT,
            "wqT": np.ascontiguousarray(np.asarray(Wq)[q_rows, :].T).astype(BF16),
            "wkT": np.ascontiguousarray(np.asarray(Wk)[kv_rows, :].T).astype(BF16),
            "wvT": np.ascontiguousarray(np.asarray(Wv)[kv_rows, :].T).astype(BF16),
            "woT": np.ascontiguousarray(np.asarray(Wo)[:, q_rows].T).astype(BF16),
            "bq": np.asarray(bq, np.float32)[q_rows].reshape(-1, 1).copy(),
            "bk": np.asarray(bk, np.float32)[kv_rows].reshape(-1, 1).copy(),
            "bvb": np.tile(np.asarray(bv, np.float32)[kv_rows][None, :],
                           (128, 1)).copy(),
            "masks": masks,
        })
    return in_maps


def kernel(x, Wq, bq, Wk, bk, Wv, bv, Wo, bo):
    from concourse.bass_utils import run_bass_kernel_spmd

    nc = _get_nc(1)
    in_maps = _prep_in_maps(x, Wq, bq, Wk, bk, Wv, bv, Wo, bo)
    res = run_bass_kernel_spmd(nc, in_maps, list(range(N_CORES))).results
    y = np.zeros((S, D_MODEL), dtype=np.float32)
    for c in range(N_CORES):
        y += res[c]["y"]
    y += np.asarray(bo, np.float32)[None, :]
    return y.reshape(1, S, D_MODEL)


# revision 12
# speedup vs baseline: 1.8452x; 1.8452x over previous
"""Trainium2 Bass kernel: GQA multi-head attention (B=1, S=2048, D=2048,
16 query heads, 4 KV heads, causal) sharded over 8 NeuronCores.

Sharding: tensor-parallel over heads. Core c owns query heads {2c, 2c+1}
and KV head c//2. Each core computes its Q/K/V projections, causal
attention for its 2 heads, and a partial output projection through its
256 rows of Wo^T. The host sums the 8 partial [S, D] outputs and adds bo.

Layout notes (per core):
  - x is fed transposed (xT [D, S], bf16) so projections produce
    Q^T/K^T [dk, S] directly (lhsT = W^T chunk, rhs = xT chunk).
  - Projections run contraction-outer (j over D/128 chunks) against a
    small set of persistent PSUM accumulators so the tensor engine
    streams right behind the xT DMAs instead of waiting for all of xT.
  - V is produced in natural [S, dk] layout (lhsT = xT chunk, rhs = Wv^T),
    4 s-tiles packed per PSUM bank.
  - Attention runs in transposed layout: scores^T[k, q] = K^T_tile.T @ Q^T,
    P^T = exp(scale * scores^T) (no max subtraction; scores are O(+-9)
    for this problem's distribution), row sums via an all-ones matmul on
    the tensor engine (broadcast across partitions), normalization folded
    into the PSUM eviction of attnout^T. Both heads' chains interleave.
  - Causal masking: fully-masked 512-wide key/query blocks are skipped,
    diagonal blocks get a narrowed free dim plus a 0/1 mask multiply.
"""

import sys

if "/opt/trn_rl_repo" not in sys.path:
    sys.path.insert(0, "/opt/trn_rl_repo")

from contextlib import ExitStack

import numpy as np
import ml_dtypes

D_MODEL = 2048
S = 2048
NUM_HEADS = 16
GROUP = 4
NUM_KV = NUM_HEADS // GROUP  # 4
DK = D_MODEL // NUM_HEADS  # 128
N_CORES = 8
HPC = NUM_HEADS // N_CORES  # 2 query heads per core
KV_DIM = DK * NUM_KV  # 512
SCALE = 1.0 / float(np.sqrt(DK))
BF16 = ml_dtypes.bfloat16

NJ = D_MODEL // 128  # 16 contraction chunks
NSC = S // 512  # 4 query chunks of 512
NST = S // 128  # 16 s-tiles / k-tiles

_CACHE: dict = {}


def _build_nc(n_iters: int = 1):
    import concourse.bass as bass
    from concourse import bacc, tile, mybir

    f32 = mybir.dt.float32
    bf16 = mybir.dt.bfloat16

    nc = bacc.Bacc("TRN2", target_bir_lowering=False, debug=False,
                   num_devices=N_CORES)

    xT_d = nc.dram_tensor("xT", [D_MODEL, S], bf16, kind="ExternalInput")
    wqT_d = nc.dram_tensor("wqT", [D_MODEL, HPC * DK], bf16, kind="ExternalInput")
    wkT_d = nc.dram_tensor("wkT", [D_MODEL, DK], bf16, kind="ExternalInput")
    wvT_d = nc.dram_tensor("wvT", [D_MODEL, DK], bf16, kind="ExternalInput")
    woT_d = nc.dram_tensor("woT", [HPC * DK, D_MODEL], bf16, kind="ExternalInput")
    bq_d = nc.dram_tensor("bq", [HPC * DK, 1], f32, kind="ExternalInput")
    bk_d = nc.dram_tensor("bk", [DK, 1], f32, kind="ExternalInput")
    bv_d = nc.dram_tensor("bv", [DK, 1], f32, kind="ExternalInput")
    masks_d = nc.dram_tensor("masks", [4, 128, 512], bf16, kind="ExternalInput")
    y_d = nc.dram_tensor("y", [S, D_MODEL], f32, kind="ExternalOutput")

    with tile.TileContext(nc) as tc, ExitStack() as ctx:
        const = ctx.enter_context(tc.tile_pool(name="const", bufs=1))
        big = ctx.enter_context(tc.tile_pool(name="big", bufs=1))
        pt_pool = ctx.enter_context(tc.tile_pool(name="pt", bufs=8))
        recip_pool = ctx.enter_context(tc.tile_pool(name="recip", bufs=2))
        yev_pool = ctx.enter_context(tc.tile_pool(name="yev", bufs=4))
        ps = ctx.enter_context(
            tc.tile_pool(name="ps", bufs=8, space=bass.MemorySpace.PSUM))

        if n_iters > 1:
            hint = (mybir.EngineType.PE, mybir.EngineType.Activation,
                    mybir.EngineType.DVE, mybir.EngineType.SP)
            ctx.enter_context(tc.For_i(0, n_iters, 1, hint_engines=hint))

        # ---- constants / weights into SBUF (emitted in consumption order)
        wq_sb = const.tile([128, NJ, HPC * DK], bf16, tag="wq")
        wk_sb = const.tile([128, NJ, DK], bf16, tag="wk")
        wv_sb = const.tile([128, NJ, DK], bf16, tag="wv")
        wo_sb = const.tile([128, HPC, D_MODEL], bf16, tag="wo")
        masks_sb = const.tile([128, 4, 512], bf16, tag="masks")
        ones_sb = const.tile([128, 128], bf16, tag="ones")
        bq_sb = const.tile([128, HPC, 1], f32, tag="bq")
        bk_sb = const.tile([128, 1], f32, tag="bk")
        bv_sb = const.tile([128, 1], f32, tag="bv")
        xT_sb = big.tile([128, NJ, S], bf16, tag="xT")

        wqT_r = wqT_d[:].rearrange("(j p) d -> p j d", p=128)
        wkT_r = wkT_d[:].rearrange("(j p) d -> p j d", p=128)
        wvT_r = wvT_d[:].rearrange("(j p) d -> p j d", p=128)
        for j in range(NJ):
            nc.sync.dma_start(out=xT_sb[:, j, :],
                              in_=xT_d[j * 128:(j + 1) * 128, :])
            nc.sync.dma_start(out=wq_sb[:, j, :], in_=wqT_r[:, j, :])
            nc.sync.dma_start(out=wk_sb[:, j, :], in_=wkT_r[:, j, :])
            nc.sync.dma_start(out=wv_sb[:, j, :], in_=wvT_r[:, j, :])
        nc.vector.memset(ones_sb[:], 1.0)
        nc.sync.dma_start(
            out=bq_sb[:], in_=bq_d[:].rearrange("(h p) o -> p h o", p=128))
        nc.sync.dma_start(out=bk_sb[:], in_=bk_d[:])
        nc.sync.dma_start(out=bv_sb[:], in_=bv_d[:])
        nc.sync.dma_start(
            out=masks_sb[:], in_=masks_d[:].rearrange("r p q -> p r q"))
        nc.sync.dma_start(
            out=wo_sb[:], in_=woT_d[:].rearrange("(h p) e -> p h e", p=128))

        qT_sb = big.tile([128, HPC, S], bf16, tag="qT")
        kT_sb = big.tile([128, S], bf16, tag="kT")
        v_sb = big.tile([128, NST, DK], bf16, tag="v")
        attnT_sb = big.tile([128, HPC, S], bf16, tag="attnT")

        # ---- phase 1: projections, contraction-outer, grouped per s-chunk
        # so attention on early query chunks can start while later chunks
        # are still projecting. Each group: Q(h0), Q(h1), K, V^T.
        vT_sb = big.tile([128, S], bf16, tag="vT")
        for sc in range(NSC):
            s_lo, s_hi = sc * 512, (sc + 1) * 512
            accs = [ps.tile([128, 512], f32, tag="ps", name=f"acc{i}")
                    for i in range(4)]
            for j in range(NJ):
                nc.tensor.matmul(accs[0][:], wq_sb[:, j, 0:DK],
                                 xT_sb[:, j, s_lo:s_hi],
                                 start=(j == 0), stop=(j == NJ - 1))
                nc.tensor.matmul(accs[1][:], wq_sb[:, j, DK:2 * DK],
                                 xT_sb[:, j, s_lo:s_hi],
                                 start=(j == 0), stop=(j == NJ - 1))
                nc.tensor.matmul(accs[2][:], wk_sb[:, j, :],
                                 xT_sb[:, j, s_lo:s_hi],
                                 start=(j == 0), stop=(j == NJ - 1))
                nc.tensor.matmul(accs[3][:], wv_sb[:, j, :],
                                 xT_sb[:, j, s_lo:s_hi],
                                 start=(j == 0), stop=(j == NJ - 1))
            for h in range(HPC):
                nc.vector.tensor_scalar_add(
                    out=qT_sb[:, h, s_lo:s_hi], in0=accs[h][:],
                    scalar1=bq_sb[:, h, :])
            nc.vector.tensor_scalar_add(
                out=kT_sb[:, s_lo:s_hi], in0=accs[2][:], scalar1=bk_sb[:])
            nc.vector.tensor_scalar_add(
                out=vT_sb[:, s_lo:s_hi], in0=accs[3][:], scalar1=bv_sb[:])
            # V back to natural [S, dk] layout via DMA transpose (2B dtype)
            for st in range(sc * 4, sc * 4 + 4):
                nc.sync.dma_start(out=v_sb[:, st, :],
                                  in_=vT_sb[:, st * 128:(st + 1) * 128],
                                  transpose=True)

        # ---- phase 2+3: attention (heads interleaved), then partial Wo ----
        for qc in range(NSC):
            q_lo = qc * 512
            nkt = 4 * qc + 4  # k-tiles 0 .. 4qc+3 (rest fully masked)
            for h in range(HPC):
                avps = ps.tile([128, 512], f32, tag="ps", name=f"avps{h}")
                sps = ps.tile([128, 512], f32, tag="ps", name=f"sps{h}")
                for kt in range(nkt):
                    r = kt - 4 * qc  # >=0 on diagonal blocks
                    off = 128 * r if r > 0 else 0
                    scps = ps.tile([128, 512], f32, tag="ps")
                    nc.tensor.matmul(
                        scps[:, off:512],
                        kT_sb[:, kt * 128:(kt + 1) * 128],
                        qT_sb[:, h, q_lo + off:q_lo + 512],
                        start=True, stop=True)
                    pt = pt_pool.tile([128, 512], bf16, tag="pt")
                    nc.scalar.activation(
                        out=pt[:, off:512], in_=scps[:, off:512],
                        func=mybir.ActivationFunctionType.Exp,
                        scale=SCALE)
                    if r >= 0:
                        nc.vector.tensor_mul(
                            out=pt[:, off:512], in0=pt[:, off:512],
                            in1=masks_sb[:, r, off:512])
                    nc.tensor.matmul(
                        sps[:, off:512], ones_sb[:], pt[:, off:512],
                        start=(kt == 0), stop=(kt == nkt - 1),
                        skip_group_check=True)
                    nc.tensor.matmul(
                        avps[:, off:512], v_sb[:, kt, :], pt[:, off:512],
                        start=(kt == 0), stop=(kt == nkt - 1),
                        skip_group_check=True)
                recip = recip_pool.tile([128, 512], f32, tag="recip")
                nc.vector.reciprocal_approx_fast(out=recip[:], in_=sps[:])
                nc.vector.tensor_mul(
                    out=attnT_sb[:, h, q_lo:q_lo + 512], in0=avps[:],
                    in1=recip[:])
            # partial output projection for this chunk's 4 s-tiles
            for st in range(qc * 4, qc * 4 + 4):
                for ec in range(4):
                    yps = ps.tile([128, 512], f32, tag="ps")
                    for h in range(HPC):
                        nc.tensor.matmul(
                            yps[:],
                            attnT_sb[:, h, st * 128:(st + 1) * 128],
                            wo_sb[:, h, ec * 512:(ec + 1) * 512],
                            start=(h == 0), stop=(h == HPC - 1))
                    ysb = yev_pool.tile([128, 512], f32, tag="yev")
                    nc.vector.tensor_copy(out=ysb[:], in_=yps[:])
                    nc.sync.dma_start(
                        out=y_d[st * 128:(st + 1) * 128,
                                ec * 512:(ec + 1) * 512],
                        in_=ysb[:])

    nc.compile()
    return nc


def _get_nc(n_iters: int = 1):
    key = ("nc", n_iters)
    if key not in _CACHE:
        _CACHE[key] = _build_nc(n_iters)
    return _CACHE[key]


def _make_masks() -> np.ndarray:
    kk = np.arange(128)[:, None]
    qq = np.arange(512)[None, :]
    masks = np.zeros((4, 128, 512), dtype=np.float32)
    for r in range(4):
        masks[r] = (128 * r + kk <= qq).astype(np.float32)
    return masks.astype(BF16)


def _prep_in_maps(x, Wq, bq, Wk, bk, Wv, bv, Wo, bo):
    x = np.asarray(x, dtype=np.float32)
    xT = np.ascontiguousarray(x.reshape(S, D_MODEL).T).astype(BF16)
    masks = _make_masks()
    in_maps = []
    for c in range(N_CORES):
        kv = c // 2
        q_rows = slice(c * HPC * DK, (c + 1) * HPC * DK)
        kv_rows = slice(kv * DK, (kv + 1) * DK)
        in_maps.append({
            "xT": xT,
            "wqT": np.ascontiguousarray(np.asarray(Wq)[q_rows, :].T).astype(BF16),
            "wkT": np.ascontiguousarray(np.asarray(Wk)[kv_rows, :].T).astype(BF16),
            "wvT": np.ascontiguousarray(np.asarray(Wv)[kv_rows, :].T).astype(BF16),
            "woT": np.ascontiguousarray(np.asarray(Wo)[:, q_rows].T).astype(BF16),
            "bq": np.asarray(bq, np.float32)[q_rows].reshape(-1, 1).copy(),
            "bk": np.asarray(bk, np.float32)[kv_rows].reshape(-1, 1).copy(),
            "bv": np.asarray(bv, np.float32)[kv_rows].reshape(-1, 1).copy(),
            "masks": masks,
        })
    return in_maps


def kernel(x, Wq, bq, Wk, bk, Wv, bv, Wo, bo):
    from concourse.bass_utils import run_bass_kernel_spmd

    nc = _get_nc(1)
    in_maps = _prep_in_maps(x, Wq, bq, Wk, bk, Wv, bv, Wo, bo)
    res = run_bass_kernel_spmd(nc, in_maps, list(range(N_CORES))).results
    y = np.zeros((S, D_MODEL), dtype=np.float32)
    for c in range(N_CORES):
        y += res[c]["y"]
    y += np.asarray(bo, np.float32)[None, :]
    return y.reshape(1, S, D_MODEL)
